# revision 29
# baseline (speedup 1.0000x reference)
"""Multi-head attention (B=4, S=2048, D=1024, H=16, Dk=64) on 8 trn2 NeuronCores.

Sharding: core c handles batch b = c//2 and head-half hh = c%2 (8 heads = 512
channels).  No collectives: each core produces a partial output projection
(sum over its 8 heads); the host adds the two partials per batch plus b_o.

Per-core kernel layout (all matmuls bf16 with f32 PSUM accumulation):
  Phase A: Q^T, K^T  [c=128-part, t]  and V [t=128-part, (h, 64+1)] with a
           fused ones-column per head (gives softmax denominator for free).
  Phase B: per head pair j, per t_q block n of 512:
           S^T tiles [t_k=128, t_q=512] via row-group-packed K=64 matmuls,
           exp on ScalarE (scale=1/8 folded in) in [128, 1536] batches,
           O'^T[h] [65, 512] accumulated over 16 t_k chunks (row 64 = rowsum),
           reciprocal + partition-broadcast DMA + DVE multiply to normalize.
  Phase C: output projection Y[t_q, 1024] from O^T and w_o^T slices.
"""

import sys

import numpy as np
import ml_dtypes

if "/opt/trn_rl_repo" not in sys.path:
    sys.path.insert(0, "/opt/trn_rl_repo")

import concourse.bass as bass
import concourse.tile as tile
from concourse import mybir
from concourse.bass_utils import run_bass_kernel_spmd
import concourse.bass_utils as _bass_utils
import concourse.bass2jax as _bass2jax


def _legalize_bir_json(bir_json):
    """Walrus (this toolchain's codegen) encodes at most ONE attached sync
    wait per TPB instruction; Tile emits instructions with several.  Hoist
    all but the last wait of each non-DMA instruction into standalone
    EventSemaphore instructions right before it (same engine, so stream
    order preserves the wait semantics).  DMA waits ride in queue
    descriptors and are left untouched."""
    import json as _json

    d = _json.loads(bir_json)
    n = 0
    for fn in d.get("functions", []):
        for blk in fn.get("blocks", []):
            out = []
            for inst in blk.get("instructions", []):
                si = inst.get("sync_info")
                if si and si.get("on_wait") and len(si["on_wait"]) > 1:
                    waits = si["on_wait"]
                    for w in waits[:-1]:
                        n += 1
                        out.append(
                            {
                                "debug": inst.get("debug"),
                                "engine": inst["engine"],
                                "ins": [],
                                "outs": [],
                                "name": f"{inst['name']}-hw{n}",
                                "opcode": "EventSemaphore",
                                "sync_info": {"on_update": [], "on_wait": [w]},
                            }
                        )
                    si["on_wait"] = [waits[-1]]
                out.append(inst)
            blk["instructions"] = out
    return _json.dumps(d).encode()


_orig_compile_bir_kernel = _bass_utils.compile_bir_kernel


def _patched_compile_bir_kernel(bir_json, tmpdir, neff_name="file.neff"):
    return _orig_compile_bir_kernel(_legalize_bir_json(bir_json), tmpdir, neff_name)


_bass_utils.compile_bir_kernel = _patched_compile_bir_kernel
_bass2jax.compile_bir_kernel = _patched_compile_bir_kernel

BF16 = mybir.dt.bfloat16
F32 = mybir.dt.float32
EXP = mybir.ActivationFunctionType.Exp
PSUM = bass.MemorySpace.PSUM

B = 4          # batches (full problem)
S = 2048       # sequence length
D = 1024       # d_model
CH = 512       # channels per core (8 heads x 64)
NH = 8         # heads per core
DK = 64        # head dim
NDM = 8        # d_model chunks of 128
NCI = 4        # channel chunks of 128 (head pairs)
QB = 512       # t_q block width
NQB = S // QB  # 4
NKC = S // 128  # 16 t_k chunks
N_CORES = 8

_NC_CACHE = None


def _emit(tc, xqT, xkT, xvT, wqT, wkT, wvT, woT, bq, bk, bv, y):
    nc = tc.nc

    with tc.tile_pool(name="persist", bufs=1) as persist:
        QT = [persist.tile([128, S], BF16, tag=f"qt{i}", name=f"qt{i}") for i in range(NCI)]
        KT = [persist.tile([128, S], BF16, tag=f"kt{i}", name=f"kt{i}") for i in range(NCI)]
        V = [persist.tile([128, NH, DK + 1], BF16, tag=f"v{t}", name=f"v{t}") for t in range(NKC)]
        OT = [persist.tile([128, S], BF16, tag=f"ot{i}", name=f"ot{i}") for i in range(NCI)]

        wo_sb = persist.tile([128, NCI, D], BF16, tag="wo", name="wo")
        nc.sync.dma_start(out=wo_sb, in_=woT[:].rearrange("(n p) d -> p n d", p=128))

        bq_sb = persist.tile([128, NCI], F32, tag="bq", name="bq")
        nc.sync.dma_start(out=bq_sb, in_=bq[:].rearrange("(n p) -> p n", p=128))
        bk_sb = persist.tile([128, NCI], F32, tag="bk", name="bk")
        nc.sync.dma_start(out=bk_sb, in_=bk[:].rearrange("(n p) -> p n", p=128))

        # b_v broadcast across partitions in the V tile layout [h, 64+1];
        # slot 64 of every head holds 1.0 (ones column for the rowsum).
        bv_bc = persist.tile([128, NH, DK + 1], F32, tag="bvb", name="bvb")
        bv_r = bv[:].rearrange("(h d) -> h d", h=NH)
        nc.sync.dma_start(out=bv_bc[:, :, 0:DK], in_=bv_r.partition_broadcast(128))
        nc.vector.memset(bv_bc[:, :, DK : DK + 1], 0.0)

        # ---------------- Phase A: projections ----------------
        with (
            tc.tile_pool(name="xin", bufs=2) as xin,
            tc.tile_pool(name="win", bufs=2) as win,
            tc.tile_pool(name="pspj", bufs=4, space=PSUM) as pspj,
        ):
            for xT, wT, dst, bias_sb, is_v in (
                (xqT, wqT, QT, bq_sb, False),
                (xkT, wkT, KT, bk_sb, False),
                (xvT, wvT, V, bv_bc, True),
            ):
                w_sb = win.tile([128, NDM, CH], BF16, tag="w", name="w")
                nc.sync.dma_start(
                    out=w_sb, in_=wT[:].rearrange("(n p) c -> p n c", p=128)
                )
                x_all = xin.tile([128, NDM, S], BF16, tag="x", name="x")
                x_rr = xT[:].rearrange("(n p) s -> p n s", p=128)
                h = NDM // 2
                nc.sync.dma_start(out=x_all[:, 0:h, :], in_=x_rr[:, 0:h, :])
                nc.sync.dma_start(out=x_all[:, h:NDM, :], in_=x_rr[:, h:NDM, :])
                x_sb = [x_all[:, k, :] for k in range(NDM)]
                if not is_v:
                    for ci in range(NCI):
                        for n in range(NQB):
                            ps = pspj.tile([128, QB], F32, tag="pj", name="pj")
                            for k in range(NDM):
                                nc.tensor.matmul(
                                    ps,
                                    w_sb[:, k, 128 * ci : 128 * (ci + 1)],
                                    x_sb[k][:, QB * n : QB * (n + 1)],
                                    start=(k == 0),
                                    stop=(k == NDM - 1),
                                )
                            nc.vector.tensor_scalar_add(
                                out=dst[ci][:, QB * n : QB * (n + 1)],
                                in0=ps,
                                scalar1=bias_sb[:, ci : ci + 1],
                            )
                else:
                    for t in range(NKC):
                        ps = pspj.tile([128, QB], F32, tag="pj", name="pj")
                        for k in range(NDM):
                            nc.tensor.matmul(
                                ps,
                                x_sb[k][:, 128 * t : 128 * (t + 1)],
                                w_sb[:, k, :],
                                start=(k == 0),
                                stop=(k == NDM - 1),
                            )
                        psv = ps.rearrange("p (h d) -> p h d", h=NH)
                        nc.vector.tensor_add(
                            out=dst[t][:, :, 0:DK], in0=psv, in1=bv_bc[:, :, 0:DK]
                        )
                        nc.vector.memset(dst[t][:, :, DK : DK + 1], 1.0)

        # ---------------- Phase B: attention ----------------
        import os as _os

        if _os.environ.get("K_SKIP_ATTN"):
            for j in range(NCI):
                nc.vector.tensor_copy(out=OT[j], in_=QT[j])
            _emit_phase_c(tc, OT, wo_sb, y)
            return

        mode = _os.environ.get("K_B_MODE", "full")
        EXPB = int(_os.environ.get("K_EXPB", "3"))  # units per exp batch
        PSS_BUFS = int(_os.environ.get("K_PSS_BUFS", "2"))
        PSO_BUFS = int(_os.environ.get("K_PSO_BUFS", "2"))
        PTP_EXTRA = int(_os.environ.get("K_PTP_EXTRA", "2"))
        NU = 2 * NKC  # 32 (t_k chunk, head) units per (pair, t_q block)
        NG = (NU + EXPB - 1) // EXPB  # exp groups

        with (
            tc.tile_pool(name="pss", bufs=PSS_BUFS, space=PSUM) as pss,
            tc.tile_pool(name="pso", bufs=PSO_BUFS, space=PSUM) as pso,
            tc.tile_pool(name="ptp", bufs=NG + PTP_EXTRA) as ptp,
            tc.tile_pool(name="rrp", bufs=3) as rrp,
            tc.tile_pool(name="rdp", bufs=3, space=bass.MemorySpace.DRAM) as rdp,
            tc.tile_pool(name="otm", bufs=3) as otm,
        ):
            fused = mode == "full" and bool(_os.environ.get("K_FUSE"))
            for j in range(NCI):
                for n in range(NQB):
                    if fused:
                        # Emit each head's O'^T accumulation matmul right
                        # after its PT unit's exp, so PE alternates between
                        # score fills and O'V work instead of draining O'V
                        # in a trailing burst.
                        ps_o2 = [
                            pso.tile([65, QB], F32, tag="o", name="o")
                            for _ in range(2)
                        ]
                        started = [False, False]
                        for g in range(NG):
                            u0 = EXPB * g
                            cnt = min(EXPB, NU - u0)
                            ps = pss.tile([128, EXPB * QB], F32, tag="s", name="s")
                            for du in range(cnt):
                                u = u0 + du
                                tb, h = divmod(u, 2)
                                nc.tensor.matmul(
                                    ps[:, QB * du : QB * (du + 1)],
                                    KT[j][
                                        64 * h : 64 * (h + 1),
                                        128 * tb : 128 * (tb + 1),
                                    ],
                                    QT[j][
                                        64 * h : 64 * (h + 1), QB * n : QB * (n + 1)
                                    ],
                                    start=True,
                                    stop=True,
                                )
                            pt = ptp.tile([128, EXPB * QB], BF16, tag="pt", name="pt")
                            nc.scalar.activation(
                                out=pt[:, : QB * cnt],
                                in_=ps[:, : QB * cnt],
                                func=EXP,
                                scale=0.125,
                            )
                            for du in range(cnt):
                                u = u0 + du
                                tb, h = divmod(u, 2)
                                nc.tensor.matmul(
                                    ps_o2[h],
                                    V[tb][:, 2 * j + h, :],
                                    pt[:, QB * du : QB * (du + 1)],
                                    start=(not started[h]),
                                    stop=(tb == NKC - 1),
                                )
                                started[h] = True
                        for h in range(2):
                            ps_o = ps_o2[h]
                            r = rrp.tile([1, QB], F32, tag="r", name="r")
                            nc.vector.reciprocal(r, ps_o[64:65, :])
                            rd = rdp.tile([1, QB], F32, tag="rd", name="rd")
                            nc.sync.dma_start(out=rd, in_=r)
                            rb = rrp.tile([64, QB], F32, tag="rb", name="rb")
                            nc.sync.dma_start(
                                out=rb, in_=rd[0, :].partition_broadcast(64)
                            )
                            if h == 0:
                                nc.vector.tensor_mul(
                                    out=OT[j][0:64, QB * n : QB * (n + 1)],
                                    in0=ps_o[0:64, :],
                                    in1=rb,
                                )
                            else:
                                tmp = otm.tile([64, QB], BF16, tag="tmp", name="tmp")
                                nc.vector.tensor_mul(
                                    out=tmp, in0=ps_o[0:64, :], in1=rb
                                )
                                nc.sync.dma_start(
                                    out=OT[j][64:128, QB * n : QB * (n + 1)],
                                    in_=tmp,
                                )
                        continue
                    pt_tiles = []
                    for g in range(NG):
                        u0 = EXPB * g
                        cnt = min(EXPB, NU - u0)
                        ps = pss.tile([128, EXPB * QB], F32, tag="s", name="s")
                        for du in range(cnt):
                            u = u0 + du
                            tb, h = divmod(u, 2)
                            nc.tensor.matmul(
                                ps[:, QB * du : QB * (du + 1)],
                                KT[j][64 * h : 64 * (h + 1), 128 * tb : 128 * (tb + 1)],
                                QT[j][64 * h : 64 * (h + 1), QB * n : QB * (n + 1)],
                                start=True,
                                stop=True,
                            )
                        pt = ptp.tile([128, EXPB * QB], BF16, tag="pt", name="pt")
                        nc.scalar.activation(
                            out=pt[:, : QB * cnt],
                            in_=ps[:, : QB * cnt],
                            func=EXP,
                            scale=0.125,
                        )
                        pt_tiles.append(pt)
                    if mode == "st":
                        nc.vector.tensor_copy(
                            out=OT[j][:, QB * n : QB * (n + 1)],
                            in_=pt_tiles[0][:, 0:QB],
                        )
                        continue
                    for h in range(2):
                        hh = 2 * j + h
                        M = 64 if mode == "ov64" else 65
                        ps_o = pso.tile([65, QB], F32, tag="o", name="o")
                        for tb in range(NKC):
                            g, du = divmod(2 * tb + h, EXPB)
                            nc.tensor.matmul(
                                ps_o[0:M, :],
                                V[tb][:, hh, 0:M],
                                pt_tiles[g][:, QB * du : QB * (du + 1)],
                                start=(tb == 0),
                                stop=(tb == NKC - 1),
                            )
                        if mode in ("ov64", "ov65"):
                            if h == 0:
                                nc.vector.tensor_copy(
                                    out=OT[j][0:64, QB * n : QB * (n + 1)],
                                    in_=ps_o[0:64, :],
                                )
                            else:
                                tmp = otm.tile([64, QB], BF16, tag="tmp", name="tmp")
                                nc.vector.tensor_copy(out=tmp, in_=ps_o[0:64, :])
                                nc.sync.dma_start(
                                    out=OT[j][64:128, QB * n : QB * (n + 1)], in_=tmp
                                )
                            continue
                        r = rrp.tile([1, QB], F32, tag="r", name="r")
                        nc.vector.reciprocal(r, ps_o[64:65, :])
                        rd = rdp.tile([1, QB], F32, tag="rd", name="rd")
                        nc.sync.dma_start(out=rd, in_=r)
                        rb = rrp.tile([64, QB], F32, tag="rb", name="rb")
                        nc.sync.dma_start(
                            out=rb, in_=rd[0, :].partition_broadcast(64)
                        )
                        if h == 0:
                            nc.vector.tensor_mul(
                                out=OT[j][0:64, QB * n : QB * (n + 1)],
                                in0=ps_o[0:64, :],
                                in1=rb,
                            )
                        else:
                            tmp = otm.tile([64, QB], BF16, tag="tmp", name="tmp")
                            nc.vector.tensor_mul(out=tmp, in0=ps_o[0:64, :], in1=rb)
                            nc.sync.dma_start(
                                out=OT[j][64:128, QB * n : QB * (n + 1)], in_=tmp
                            )

        _emit_phase_c(tc, OT, wo_sb, y)


def _emit_chain(tc, chain_in, chain_out):
    # Tiny passthrough used by the benchmark to serialize back-to-back NEFF
    # executions with a data dependency; ~2 DMAs, negligible cost.
    nc = tc.nc
    with tc.tile_pool(name="chp", bufs=1) as chp:
        ct = chp.tile([1, 1], F32, name="ct")
        nc.sync.dma_start(out=ct, in_=chain_in[:])
        nc.sync.dma_start(out=chain_out[:], in_=ct)


def _emit_phase_c(tc, OT, wo_sb, y):
    nc = tc.nc
    with (
        tc.tile_pool(name="psy", bufs=4, space=PSUM) as psy,
        tc.tile_pool(name="yp", bufs=3) as yp,
    ):
        y_r = y[:].rearrange("(a p) d -> a p d", p=128)
        for m in range(S // 128):
            y_sb = yp.tile([128, D], F32, tag="y", name="y")
            for half in range(2):
                ps = psy.tile([128, QB], F32, tag="ps", name="ps")
                for j in range(NCI):
                    nc.tensor.matmul(
                        ps,
                        OT[j][:, 128 * m : 128 * (m + 1)],
                        wo_sb[:, j, QB * half : QB * (half + 1)],
                        start=(j == 0),
                        stop=(j == NCI - 1),
                    )
                nc.vector.tensor_copy(y_sb[:, QB * half : QB * (half + 1)], ps)
            nc.sync.dma_start(out=y_r[m], in_=y_sb)


def build_nc():
    nc = bass.Bass(target_bir_lowering=False)
    xqT = nc.declare_dram_parameter("xqT", [D, S], BF16, isOutput=False)
    xkT = nc.declare_dram_parameter("xkT", [D, S], BF16, isOutput=False)
    xvT = nc.declare_dram_parameter("xvT", [D, S], BF16, isOutput=False)
    wqT = nc.declare_dram_parameter("wqT", [D, CH], BF16, isOutput=False)
    wkT = nc.declare_dram_parameter("wkT", [D, CH], BF16, isOutput=False)
    wvT = nc.declare_dram_parameter("wvT", [D, CH], BF16, isOutput=False)
    woT = nc.declare_dram_parameter("woT", [CH, D], BF16, isOutput=False)
    bq = nc.declare_dram_parameter("bq", [CH], F32, isOutput=False)
    bk = nc.declare_dram_parameter("bk", [CH], F32, isOutput=False)
    bv = nc.declare_dram_parameter("bv", [CH], F32, isOutput=False)
    y = nc.declare_dram_parameter("y", [S, D], F32, isOutput=True)
    chain_in = nc.declare_dram_parameter("chain_in", [1, 1], F32, isOutput=False)
    chain_out = nc.declare_dram_parameter("chain_out", [1, 1], F32, isOutput=True)
    with tile.TileContext(nc) as tc:
        _emit(tc, xqT, xkT, xvT, wqT, wkT, wvT, woT, bq, bk, bv, y)
        _emit_chain(tc, chain_in, chain_out)
    nc.finalize()
    return nc


def make_in_maps(query, key, value, w_q, b_q, w_k, b_k, w_v, b_v, w_o, b_o):
    bf = ml_dtypes.bfloat16
    query = np.asarray(query, np.float32)
    key = np.asarray(key, np.float32)
    value = np.asarray(value, np.float32)
    w_q = np.asarray(w_q, np.float32)
    w_k = np.asarray(w_k, np.float32)
    w_v = np.asarray(w_v, np.float32)
    w_o = np.asarray(w_o, np.float32)
    in_maps = []
    for c in range(N_CORES):
        b, hh = divmod(c, 2)
        sl = slice(hh * CH, (hh + 1) * CH)
        in_maps.append(
            {
                "xqT": query[b].T.astype(bf),
                "xkT": key[b].T.astype(bf),
                "xvT": value[b].T.astype(bf),
                "wqT": w_q[sl].T.astype(bf),
                "wkT": w_k[sl].T.astype(bf),
                "wvT": w_v[sl].T.astype(bf),
                "woT": w_o[:, sl].T.astype(bf),
                "bq": np.ascontiguousarray(np.asarray(b_q, np.float32)[sl]),
                "bk": np.ascontiguousarray(np.asarray(b_k, np.float32)[sl]),
                "bv": np.ascontiguousarray(np.asarray(b_v, np.float32)[sl]),
                "chain_in": np.zeros((1, 1), np.float32),
            }
        )
    return in_maps


def run(trace=False, **inputs):
    global _NC_CACHE
    if _NC_CACHE is None:
        _NC_CACHE = build_nc()
    in_maps = make_in_maps(**inputs)
    res = run_bass_kernel_spmd(_NC_CACHE, in_maps, list(range(N_CORES)), trace=trace)
    b_o = np.asarray(inputs["b_o"], np.float32)
    y = np.empty((B, S, D), np.float32)
    for b in range(B):
        y[b] = res.results[2 * b]["y"] + res.results[2 * b + 1]["y"] + b_o
    return y, res


def kernel(**inputs):
    y, _ = run(trace=False, **inputs)
    return y


# revision 30
# speedup vs baseline: 1.0190x; 1.0190x over previous
"""Multi-head attention (B=4, S=2048, D=1024, H=16, Dk=64) on 8 trn2 NeuronCores.

Sharding: core c handles batch b = c//2 and head-half hh = c%2 (8 heads = 512
channels).  No collectives: each core produces a partial output projection
(sum over its 8 heads); the host adds the two partials per batch plus b_o.

Per-core kernel layout (all matmuls bf16 with f32 PSUM accumulation):
  Phase A: Q^T, K^T  [c=128-part, t]  and V [t=128-part, (h, 64+1)] with a
           fused ones-column per head (gives softmax denominator for free).
  Phase B: per head pair j, per t_q block n of 512:
           S^T tiles [t_k=128, t_q=512] via row-group-packed K=64 matmuls,
           exp on ScalarE (scale=1/8 folded in) in [128, 1536] batches,
           O'^T[h] [65, 512] accumulated over 16 t_k chunks (row 64 = rowsum),
           reciprocal + partition-broadcast DMA + DVE multiply to normalize.
  Phase C: output projection Y[t_q, 1024] from O^T and w_o^T slices.
"""

import sys

import numpy as np
import ml_dtypes

if "/opt/trn_rl_repo" not in sys.path:
    sys.path.insert(0, "/opt/trn_rl_repo")

import concourse.bass as bass
import concourse.tile as tile
from concourse import mybir
from concourse.bass_utils import run_bass_kernel_spmd
import concourse.bass_utils as _bass_utils
import concourse.bass2jax as _bass2jax


def _legalize_bir_json(bir_json):
    """Walrus (this toolchain's codegen) encodes at most ONE attached sync
    wait per TPB instruction; Tile emits instructions with several.  Hoist
    all but the last wait of each non-DMA instruction into standalone
    EventSemaphore instructions right before it (same engine, so stream
    order preserves the wait semantics).  DMA waits ride in queue
    descriptors and are left untouched."""
    import json as _json

    d = _json.loads(bir_json)
    n = 0
    for fn in d.get("functions", []):
        for blk in fn.get("blocks", []):
            out = []
            for inst in blk.get("instructions", []):
                si = inst.get("sync_info")
                if si and si.get("on_wait") and len(si["on_wait"]) > 1:
                    waits = si["on_wait"]
                    for w in waits[:-1]:
                        n += 1
                        out.append(
                            {
                                "debug": inst.get("debug"),
                                "engine": inst["engine"],
                                "ins": [],
                                "outs": [],
                                "name": f"{inst['name']}-hw{n}",
                                "opcode": "EventSemaphore",
                                "sync_info": {"on_update": [], "on_wait": [w]},
                            }
                        )
                    si["on_wait"] = [waits[-1]]
                out.append(inst)
            blk["instructions"] = out
    return _json.dumps(d).encode()


_orig_compile_bir_kernel = _bass_utils.compile_bir_kernel


def _patched_compile_bir_kernel(bir_json, tmpdir, neff_name="file.neff"):
    return _orig_compile_bir_kernel(_legalize_bir_json(bir_json), tmpdir, neff_name)


_bass_utils.compile_bir_kernel = _patched_compile_bir_kernel
_bass2jax.compile_bir_kernel = _patched_compile_bir_kernel

BF16 = mybir.dt.bfloat16
F32 = mybir.dt.float32
EXP = mybir.ActivationFunctionType.Exp
PSUM = bass.MemorySpace.PSUM

B = 4          # batches (full problem)
S = 2048       # sequence length
D = 1024       # d_model
CH = 512       # channels per core (8 heads x 64)
NH = 8         # heads per core
DK = 64        # head dim
NDM = 8        # d_model chunks of 128
NCI = 4        # channel chunks of 128 (head pairs)
QB = 512       # t_q block width
NQB = S // QB  # 4
NKC = S // 128  # 16 t_k chunks
N_CORES = 8

_NC_CACHE = None


def _emit(tc, xqT, xkT, xvT, wqT, wkT, wvT, woT, bq, bk, bv, y):
    nc = tc.nc

    with tc.tile_pool(name="persist", bufs=1) as persist:
        QT = [persist.tile([128, S], BF16, tag=f"qt{i}", name=f"qt{i}") for i in range(NCI)]
        KT = [persist.tile([128, S], BF16, tag=f"kt{i}", name=f"kt{i}") for i in range(NCI)]
        V = [persist.tile([128, NH, DK + 1], BF16, tag=f"v{t}", name=f"v{t}") for t in range(NKC)]
        OT = [persist.tile([128, S], BF16, tag=f"ot{i}", name=f"ot{i}") for i in range(NCI)]

        wo_sb = persist.tile([128, NCI, D], BF16, tag="wo", name="wo")
        nc.sync.dma_start(out=wo_sb, in_=woT[:].rearrange("(n p) d -> p n d", p=128))

        bq_sb = persist.tile([128, NCI], F32, tag="bq", name="bq")
        nc.sync.dma_start(out=bq_sb, in_=bq[:].rearrange("(n p) -> p n", p=128))
        bk_sb = persist.tile([128, NCI], F32, tag="bk", name="bk")
        nc.sync.dma_start(out=bk_sb, in_=bk[:].rearrange("(n p) -> p n", p=128))

        # b_v broadcast across partitions in the V tile layout [h, 64+1];
        # slot 64 of every head holds 1.0 (ones column for the rowsum).
        bv_bc = persist.tile([128, NH, DK + 1], F32, tag="bvb", name="bvb")
        bv_r = bv[:].rearrange("(h d) -> h d", h=NH)
        nc.sync.dma_start(out=bv_bc[:, :, 0:DK], in_=bv_r.partition_broadcast(128))
        nc.vector.memset(bv_bc[:, :, DK : DK + 1], 0.0)

        # ---------------- Phase A: projections ----------------
        with (
            tc.tile_pool(name="xin", bufs=2) as xin,
            tc.tile_pool(name="win", bufs=2) as win,
            tc.tile_pool(name="pspj", bufs=4, space=PSUM) as pspj,
        ):
            for xT, wT, dst, bias_sb, is_v in (
                (xqT, wqT, QT, bq_sb, False),
                (xkT, wkT, KT, bk_sb, False),
                (xvT, wvT, V, bv_bc, True),
            ):
                w_sb = win.tile([128, NDM, CH], BF16, tag="w", name="w")
                nc.sync.dma_start(
                    out=w_sb, in_=wT[:].rearrange("(n p) c -> p n c", p=128)
                )
                x_all = xin.tile([128, NDM, S], BF16, tag="x", name="x")
                x_rr = xT[:].rearrange("(n p) s -> p n s", p=128)
                h = NDM // 2
                nc.sync.dma_start(out=x_all[:, 0:1, :], in_=x_rr[:, 0:1, :])
                nc.sync.dma_start(out=x_all[:, 1:h, :], in_=x_rr[:, 1:h, :])
                nc.sync.dma_start(out=x_all[:, h:NDM, :], in_=x_rr[:, h:NDM, :])
                x_sb = [x_all[:, k, :] for k in range(NDM)]
                if not is_v:
                    for ci in range(NCI):
                        for n in range(NQB):
                            ps = pspj.tile([128, QB], F32, tag="pj", name="pj")
                            for k in range(NDM):
                                nc.tensor.matmul(
                                    ps,
                                    w_sb[:, k, 128 * ci : 128 * (ci + 1)],
                                    x_sb[k][:, QB * n : QB * (n + 1)],
                                    start=(k == 0),
                                    stop=(k == NDM - 1),
                                )
                            nc.vector.tensor_scalar_add(
                                out=dst[ci][:, QB * n : QB * (n + 1)],
                                in0=ps,
                                scalar1=bias_sb[:, ci : ci + 1],
                            )
                else:
                    for t in range(NKC):
                        ps = pspj.tile([128, QB], F32, tag="pj", name="pj")
                        for k in range(NDM):
                            nc.tensor.matmul(
                                ps,
                                x_sb[k][:, 128 * t : 128 * (t + 1)],
                                w_sb[:, k, :],
                                start=(k == 0),
                                stop=(k == NDM - 1),
                            )
                        psv = ps.rearrange("p (h d) -> p h d", h=NH)
                        nc.vector.tensor_add(
                            out=dst[t][:, :, 0:DK], in0=psv, in1=bv_bc[:, :, 0:DK]
                        )
                        nc.vector.memset(dst[t][:, :, DK : DK + 1], 1.0)

        # ---------------- Phase B: attention ----------------
        import os as _os

        if _os.environ.get("K_SKIP_ATTN"):
            for j in range(NCI):
                nc.vector.tensor_copy(out=OT[j], in_=QT[j])
            _emit_phase_c(tc, OT, wo_sb, y)
            return

        mode = _os.environ.get("K_B_MODE", "full")
        EXPB = int(_os.environ.get("K_EXPB", "3"))  # units per exp batch
        PSS_BUFS = int(_os.environ.get("K_PSS_BUFS", "2"))
        PSO_BUFS = int(_os.environ.get("K_PSO_BUFS", "2"))
        PTP_EXTRA = int(_os.environ.get("K_PTP_EXTRA", "2"))
        NU = 2 * NKC  # 32 (t_k chunk, head) units per (pair, t_q block)
        NG = (NU + EXPB - 1) // EXPB  # exp groups

        with (
            tc.tile_pool(name="pss", bufs=PSS_BUFS, space=PSUM) as pss,
            tc.tile_pool(name="pso", bufs=PSO_BUFS, space=PSUM) as pso,
            tc.tile_pool(name="ptp", bufs=NG + PTP_EXTRA) as ptp,
            tc.tile_pool(name="rrp", bufs=3) as rrp,
            tc.tile_pool(name="rdp", bufs=3, space=bass.MemorySpace.DRAM) as rdp,
            tc.tile_pool(name="otm", bufs=3) as otm,
        ):
            fused = mode == "full" and bool(_os.environ.get("K_FUSE"))
            for j in range(NCI):
                for n in range(NQB):
                    if fused:
                        # Emit each head's O'^T accumulation matmul right
                        # after its PT unit's exp, so PE alternates between
                        # score fills and O'V work instead of draining O'V
                        # in a trailing burst.
                        ps_o2 = [
                            pso.tile([65, QB], F32, tag="o", name="o")
                            for _ in range(2)
                        ]
                        started = [False, False]
                        for g in range(NG):
                            u0 = EXPB * g
                            cnt = min(EXPB, NU - u0)
                            ps = pss.tile([128, EXPB * QB], F32, tag="s", name="s")
                            for du in range(cnt):
                                u = u0 + du
                                tb, h = divmod(u, 2)
                                nc.tensor.matmul(
                                    ps[:, QB * du : QB * (du + 1)],
                                    KT[j][
                                        64 * h : 64 * (h + 1),
                                        128 * tb : 128 * (tb + 1),
                                    ],
                                    QT[j][
                                        64 * h : 64 * (h + 1), QB * n : QB * (n + 1)
                                    ],
                                    start=True,
                                    stop=True,
                                )
                            pt = ptp.tile([128, EXPB * QB], BF16, tag="pt", name="pt")
                            nc.scalar.activation(
                                out=pt[:, : QB * cnt],
                                in_=ps[:, : QB * cnt],
                                func=EXP,
                                scale=0.125,
                            )
                            for du in range(cnt):
                                u = u0 + du
                                tb, h = divmod(u, 2)
                                nc.tensor.matmul(
                                    ps_o2[h],
                                    V[tb][:, 2 * j + h, :],
                                    pt[:, QB * du : QB * (du + 1)],
                                    start=(not started[h]),
                                    stop=(tb == NKC - 1),
                                )
                                started[h] = True
                        for h in range(2):
                            ps_o = ps_o2[h]
                            r = rrp.tile([1, QB], F32, tag="r", name="r")
                            nc.vector.reciprocal(r, ps_o[64:65, :])
                            rd = rdp.tile([1, QB], F32, tag="rd", name="rd")
                            nc.sync.dma_start(out=rd, in_=r)
                            rb = rrp.tile([64, QB], F32, tag="rb", name="rb")
                            nc.sync.dma_start(
                                out=rb, in_=rd[0, :].partition_broadcast(64)
                            )
                            if h == 0:
                                nc.vector.tensor_mul(
                                    out=OT[j][0:64, QB * n : QB * (n + 1)],
                                    in0=ps_o[0:64, :],
                                    in1=rb,
                                )
                            else:
                                tmp = otm.tile([64, QB], BF16, tag="tmp", name="tmp")
                                nc.vector.tensor_mul(
                                    out=tmp, in0=ps_o[0:64, :], in1=rb
                                )
                                nc.sync.dma_start(
                                    out=OT[j][64:128, QB * n : QB * (n + 1)],
                                    in_=tmp,
                                )
                        continue
                    pt_tiles = []
                    for g in range(NG):
                        u0 = EXPB * g
                        cnt = min(EXPB, NU - u0)
                        ps = pss.tile([128, EXPB * QB], F32, tag="s", name="s")
                        for du in range(cnt):
                            u = u0 + du
                            tb, h = divmod(u, 2)
                            nc.tensor.matmul(
                                ps[:, QB * du : QB * (du + 1)],
                                KT[j][64 * h : 64 * (h + 1), 128 * tb : 128 * (tb + 1)],
                                QT[j][64 * h : 64 * (h + 1), QB * n : QB * (n + 1)],
                                start=True,
                                stop=True,
                            )
                        pt = ptp.tile([128, EXPB * QB], BF16, tag="pt", name="pt")
                        nc.scalar.activation(
                            out=pt[:, : QB * cnt],
                            in_=ps[:, : QB * cnt],
                            func=EXP,
                            scale=0.125,
                        )
                        pt_tiles.append(pt)
                    if mode == "st":
                        nc.vector.tensor_copy(
                            out=OT[j][:, QB * n : QB * (n + 1)],
                            in_=pt_tiles[0][:, 0:QB],
                        )
                        continue
                    for h in range(2):
                        hh = 2 * j + h
                        M = 64 if mode == "ov64" else 65
                        ps_o = pso.tile([65, QB], F32, tag="o", name="o")
                        for tb in range(NKC):
                            g, du = divmod(2 * tb + h, EXPB)
                            nc.tensor.matmul(
                                ps_o[0:M, :],
                                V[tb][:, hh, 0:M],
                                pt_tiles[g][:, QB * du : QB * (du + 1)],
                                start=(tb == 0),
                                stop=(tb == NKC - 1),
                            )
                        if mode in ("ov64", "ov65"):
                            if h == 0:
                                nc.vector.tensor_copy(
                                    out=OT[j][0:64, QB * n : QB * (n + 1)],
                                    in_=ps_o[0:64, :],
                                )
                            else:
                                tmp = otm.tile([64, QB], BF16, tag="tmp", name="tmp")
                                nc.vector.tensor_copy(out=tmp, in_=ps_o[0:64, :])
                                nc.sync.dma_start(
                                    out=OT[j][64:128, QB * n : QB * (n + 1)], in_=tmp
                                )
                            continue
                        r = rrp.tile([1, QB], F32, tag="r", name="r")
                        nc.vector.reciprocal(r, ps_o[64:65, :])
                        rd = rdp.tile([1, QB], F32, tag="rd", name="rd")
                        nc.sync.dma_start(out=rd, in_=r)
                        rb = rrp.tile([64, QB], F32, tag="rb", name="rb")
                        nc.sync.dma_start(
                            out=rb, in_=rd[0, :].partition_broadcast(64)
                        )
                        if h == 0:
                            nc.vector.tensor_mul(
                                out=OT[j][0:64, QB * n : QB * (n + 1)],
                                in0=ps_o[0:64, :],
                                in1=rb,
                            )
                        else:
                            tmp = otm.tile([64, QB], BF16, tag="tmp", name="tmp")
                            nc.vector.tensor_mul(out=tmp, in0=ps_o[0:64, :], in1=rb)
                            nc.sync.dma_start(
                                out=OT[j][64:128, QB * n : QB * (n + 1)], in_=tmp
                            )

        _emit_phase_c(tc, OT, wo_sb, y)


def _emit_chain(tc, chain_in, chain_out):
    # Tiny passthrough used by the benchmark to serialize back-to-back NEFF
    # executions with a data dependency; ~2 DMAs, negligible cost.
    nc = tc.nc
    with tc.tile_pool(name="chp", bufs=1) as chp:
        ct = chp.tile([1, 1], F32, name="ct")
        nc.sync.dma_start(out=ct, in_=chain_in[:])
        nc.sync.dma_start(out=chain_out[:], in_=ct)


def _emit_phase_c(tc, OT, wo_sb, y):
    nc = tc.nc
    with (
        tc.tile_pool(name="psy", bufs=4, space=PSUM) as psy,
        tc.tile_pool(name="yp", bufs=3) as yp,
    ):
        y_r = y[:].rearrange("(a p) d -> a p d", p=128)
        for m in range(S // 128):
            y_sb = yp.tile([128, D], F32, tag="y", name="y")
            for half in range(2):
                ps = psy.tile([128, QB], F32, tag="ps", name="ps")
                for j in range(NCI):
                    nc.tensor.matmul(
                        ps,
                        OT[j][:, 128 * m : 128 * (m + 1)],
                        wo_sb[:, j, QB * half : QB * (half + 1)],
                        start=(j == 0),
                        stop=(j == NCI - 1),
                    )
                nc.vector.tensor_copy(y_sb[:, QB * half : QB * (half + 1)], ps)
            nc.sync.dma_start(out=y_r[m], in_=y_sb)


def build_nc():
    nc = bass.Bass(target_bir_lowering=False)
    xqT = nc.declare_dram_parameter("xqT", [D, S], BF16, isOutput=False)
    xkT = nc.declare_dram_parameter("xkT", [D, S], BF16, isOutput=False)
    xvT = nc.declare_dram_parameter("xvT", [D, S], BF16, isOutput=False)
    wqT = nc.declare_dram_parameter("wqT", [D, CH], BF16, isOutput=False)
    wkT = nc.declare_dram_parameter("wkT", [D, CH], BF16, isOutput=False)
    wvT = nc.declare_dram_parameter("wvT", [D, CH], BF16, isOutput=False)
    woT = nc.declare_dram_parameter("woT", [CH, D], BF16, isOutput=False)
    bq = nc.declare_dram_parameter("bq", [CH], F32, isOutput=False)
    bk = nc.declare_dram_parameter("bk", [CH], F32, isOutput=False)
    bv = nc.declare_dram_parameter("bv", [CH], F32, isOutput=False)
    y = nc.declare_dram_parameter("y", [S, D], F32, isOutput=True)
    chain_in = nc.declare_dram_parameter("chain_in", [1, 1], F32, isOutput=False)
    chain_out = nc.declare_dram_parameter("chain_out", [1, 1], F32, isOutput=True)
    with tile.TileContext(nc) as tc:
        _emit(tc, xqT, xkT, xvT, wqT, wkT, wvT, woT, bq, bk, bv, y)
        _emit_chain(tc, chain_in, chain_out)
    nc.finalize()
    return nc


def make_in_maps(query, key, value, w_q, b_q, w_k, b_k, w_v, b_v, w_o, b_o):
    bf = ml_dtypes.bfloat16
    query = np.asarray(query, np.float32)
    key = np.asarray(key, np.float32)
    value = np.asarray(value, np.float32)
    w_q = np.asarray(w_q, np.float32)
    w_k = np.asarray(w_k, np.float32)
    w_v = np.asarray(w_v, np.float32)
    w_o = np.asarray(w_o, np.float32)
    in_maps = []
    for c in range(N_CORES):
        b, hh = divmod(c, 2)
        sl = slice(hh * CH, (hh + 1) * CH)
        in_maps.append(
            {
                "xqT": query[b].T.astype(bf),
                "xkT": key[b].T.astype(bf),
                "xvT": value[b].T.astype(bf),
                "wqT": w_q[sl].T.astype(bf),
                "wkT": w_k[sl].T.astype(bf),
                "wvT": w_v[sl].T.astype(bf),
                "woT": w_o[:, sl].T.astype(bf),
                "bq": np.ascontiguousarray(np.asarray(b_q, np.float32)[sl]),
                "bk": np.ascontiguousarray(np.asarray(b_k, np.float32)[sl]),
                "bv": np.ascontiguousarray(np.asarray(b_v, np.float32)[sl]),
                "chain_in": np.zeros((1, 1), np.float32),
            }
        )
    return in_maps


def run(trace=False, **inputs):
    global _NC_CACHE
    if _NC_CACHE is None:
        _NC_CACHE = build_nc()
    in_maps = make_in_maps(**inputs)
    res = run_bass_kernel_spmd(_NC_CACHE, in_maps, list(range(N_CORES)), trace=trace)
    b_o = np.asarray(inputs["b_o"], np.float32)
    y = np.empty((B, S, D), np.float32)
    for b in range(B):
        y[b] = res.results[2 * b]["y"] + res.results[2 * b + 1]["y"] + b_o
    return y, res


def kernel(**inputs):
    y, _ = run(trace=False, **inputs)
    return y
